# revision 1
# baseline (speedup 1.0000x reference)
"""Single-head causal attention on 8 trn2 NeuronCores.

Problem: x:[4,4096,1024] f32; Wk/Wq/Wv:[1024,64].
  q,k,v = x@W*; S = q k^T / 8 causal-masked; out = softmax(S) @ v.

Sharding: 2 cores per batch (8 = 4 batches x 2 roles). Each core handles 8
"q-supers" of 256 queries, interleaved so causal work balances across the
role pair. kv is computed over the full batch on both cores (duplicated —
no collectives). SPMD: one program, per-core data (x slice, schedule,
masks, role) makes the cores differ.

Per-core layout trick: scores are computed transposed (S^T[s,q]) with
K^T/Q^T held H-on-partition and duplicated across both 64-partition halves
so two key-blocks run concurrently via PE row-tiling. exp(S^T) on ScalarE
(scale=1/8 fused). AV uses V natural [s,h+1] (ones column => row-sums ride
along) producing O^T[h+1, q], transposed back on PE and divided by the
row-sums at the end. No online-softmax max-subtraction: scores are ~N(0,1)
(max |s| < 7 for these inputs), exp is safe in fp32.
"""

import numpy as np

B, T, C, H = 4, 4096, 1024, 64
NCORES = 8
SUP = 256            # q-super size
NSLOTS = 8           # q-supers per core
NSUP = T // SUP      # 16 q-supers per batch
E_PAD = [2, 16, 4, 14, 6, 12, 8, 10]          # padded s-extent per slot (supers)
POS = [
    [0, 15, 2, 13, 4, 11, 6, 9],              # role 0 q-super positions
    [1, 14, 3, 12, 5, 10, 7, 8],              # role 1
]
SCALE = 0.125        # 1/sqrt(64)

# fp32r ("replicated" fp32, 4x matmul throughput at N>=256) toggles.
import os as _os

_F = _os.environ.get("F32R", "")   # letters: p=proj, s=scores, a=AV, t=transposes
F32R_PROJ = "p" in _F
F32R_SCORES = "s" in _F
F32R_AV = "a" in _F
F32R_TR = "t" in _F

_CACHE = {}


def _masks(role):
    """(mask_even, mask_odd) [128, 4, SUP] f32 multiplicative masks for the
    last 4 key-blocks of every slot. 'far' = diagonal in window blocks 0,1
    (blocks 2,3 are padding overshoot -> zero); 'near' = diagonal in blocks
    2,3 (blocks 0,1 fully allowed)."""
    ps = np.arange(128)[:, None]
    f = np.arange(SUP)[None, :]
    tri0 = (f >= ps).astype(np.float32)
    tri1 = (f >= ps + 128).astype(np.float32)
    far = np.stack([tri0, tri1, np.zeros_like(tri0), np.zeros_like(tri0)], 0)
    near = np.stack([np.ones_like(tri0), np.ones_like(tri0), tri0, tri1], 0)
    out = []
    for parity in (0, 1):
        m = far if parity == role else near
        out.append(np.ascontiguousarray(m.transpose(1, 0, 2)))  # [128, 4, SUP]
    return out


def _build():
    import concourse.tile as tile
    from concourse import bacc, mybir
    from concourse.bass import ds

    dt = mybir.dt
    f32 = dt.float32
    f32r = dt.float32r
    d_proj = f32r if F32R_PROJ else f32      # W packs, xT
    d_sc = f32r if F32R_SCORES else f32      # kt_dup, qt_pos, qt_slot
    d_av = f32r if F32R_AV else f32          # v_aug, P tiles, masks

    nc = bacc.Bacc(
        "TRN2",
        target_bir_lowering=False,
        debug=False,
        enable_asserts=False,
        num_devices=NCORES,
    )

    x_d = nc.dram_tensor("x", [T, C], f32, kind="ExternalInput").ap()
    wq_d = nc.dram_tensor("wq", [C, H], d_proj, kind="ExternalInput").ap()
    wk_d = nc.dram_tensor("wk", [C, H], d_proj, kind="ExternalInput").ap()
    wv_d = nc.dram_tensor("wv", [C, H], d_proj, kind="ExternalInput").ap()
    id_d = nc.dram_tensor("ident", [128, 128], f32, kind="ExternalInput").ap()
    me_d = nc.dram_tensor("mask_even", [128, 4, SUP], d_av, kind="ExternalInput").ap()
    mo_d = nc.dram_tensor("mask_odd", [128, 4, SUP], d_av, kind="ExternalInput").ap()
    sc_d = nc.dram_tensor("sched", [1, NSLOTS], dt.int32, kind="ExternalInput").ap()
    out_d = nc.dram_tensor("out", [NSLOTS * SUP, H], f32, kind="ExternalOutput").ap()

    def mmcast(ap, on):
        return ap.bitcast(f32r) if on else ap

    def tr(out, in_, idn):
        if F32R_TR:
            nc.tensor.matmul(out.bitcast(f32r), in_.bitcast(f32r),
                             idn.bitcast(f32r), is_transpose=True)
        else:
            nc.tensor.transpose(out, in_, idn)

    with tile.TileContext(nc) as tc:
        with tc.tile_pool(name="const", bufs=1) as const, \
             tc.tile_pool(name="persist", bufs=1) as persist:
            ident = const.tile([128, 128], f32)
            nc.sync.dma_start(ident, id_d)
            m_ev = const.tile([128, 4, SUP], d_av)
            nc.sync.dma_start(m_ev, me_d)
            m_od = const.tile([128, 4, SUP], d_av)
            nc.sync.dma_start(m_od, mo_d)
            wqk = const.tile([128, 8, 128], d_proj)
            nc.sync.dma_start(
                wqk[:, :, 0:H], wq_d.rearrange("(cb p) h -> p cb h", p=128))
            nc.sync.dma_start(
                wqk[:, :, H:128], wk_d.rearrange("(cb p) h -> p cb h", p=128))
            wvt = const.tile([128, 8, H], d_proj)
            nc.sync.dma_start(wvt, wv_d.rearrange("(cb p) h -> p cb h", p=128))
            sched = const.tile([1, NSLOTS], dt.int32)
            nc.sync.dma_start(sched, sc_d)

            qt_pos = persist.tile([64, T], d_sc)       # Q^T position-ordered
            kt_dup = persist.tile([128, T], d_sc)      # K^T on both halves
            qt_slot = persist.tile([128, NSLOTS * SUP], d_sc)
            v_aug = persist.tile([128, T // 128, H + 1], d_av)
            if d_av == f32:
                nc.gpsimd.memset(v_aug[:, :, H : H + 1], 1.0)
            else:
                ones_st = const.tile([128, T // 128], f32)
                nc.gpsimd.memset(ones_st, 1.0)
                nc.vector.tensor_copy(v_aug[:, :, H], ones_st)

            # ---- Phase 1: stream x, transpose, project ----
            with tc.tile_pool(name="xt", bufs=2) as xtp, \
                 tc.tile_pool(name="xT", bufs=2) as xTp, \
                 tc.tile_pool(name="vts", bufs=2) as vtsp, \
                 tc.tile_pool(name="tps", bufs=2, space="PSUM") as tpp, \
                 tc.tile_pool(name="qkp", bufs=2, space="PSUM") as qkpp, \
                 tc.tile_pool(name="vtp", bufs=2, space="PSUM") as vtpp:
                for ch in range(T // 512):
                    cs = slice(ch * 512, (ch + 1) * 512)
                    xt = xtp.tile([128, 4, C], f32)
                    nc.sync.dma_start(
                        xt, x_d[cs, :].rearrange("(tb p) c -> p tb c", p=128))
                    xT = xTp.tile([128, 8, 512], d_proj)
                    for tb in range(4):
                        for cb in range(8):
                            tp = tpp.tile([128, 128], f32, tag='tp')
                            tr(tp, xt[:, tb, cb * 128 : (cb + 1) * 128], ident)
                            dst = xT[:, cb, tb * 128 : (tb + 1) * 128]
                            if cb % 2 == 0:
                                nc.vector.tensor_copy(dst, tp)
                            else:
                                nc.scalar.copy(dst, tp)
                    qk = qkpp.tile([128, 512], f32)
                    for cb in range(8):
                        nc.tensor.matmul(
                            qk, wqk[:, cb, :], xT[:, cb, :],
                            start=(cb == 0), stop=(cb == 7))
                    vt = vtpp.tile([64, 512], f32)
                    for cb in range(8):
                        nc.tensor.matmul(
                            vt, wvt[:, cb, :], xT[:, cb, :],
                            start=(cb == 0), stop=(cb == 7))
                    nc.scalar.copy(qt_pos[:, cs], qk[0:64, :])
                    nc.vector.tensor_copy(kt_dup[64:128, cs], qk[64:128, :])
                    vts = vtsp.tile([64, 512], f32)
                    nc.scalar.copy(vts, vt)
                    for tb in range(4):
                        vp = tpp.tile([128, 128], f32, tag='tp')
                        tr(vp[:, 0:H], vts[:, tb * 128 : (tb + 1) * 128],
                           ident[0:64, 0:64])
                        nc.vector.tensor_copy(
                            v_aug[:, ch * 4 + tb, 0:H], vp[:, 0:H])

                # ---- Phase 1.5: duplicate K^T, permute+duplicate Q^T ----
                nc.gpsimd.dma_start(kt_dup[0:64, :], kt_dup[64:128, :])
                _, vals = nc.values_load_multi_w_load_instructions(
                    sched[0:1, :], engines=[mybir.EngineType.Pool],
                    min_val=0, max_val=(NSUP - 1) * SUP,
                    skip_runtime_bounds_check=True)
                for j in range(NSLOTS):
                    nc.gpsimd.dma_start(
                        qt_slot[0:64, j * SUP : (j + 1) * SUP],
                        qt_pos[0:64, ds(vals[j], SUP)])
                nc.gpsimd.dma_start(qt_slot[64:128, :], qt_slot[0:64, :])

            # ---- Phase 2: attention ----
            with tc.tile_pool(name="pt", bufs=6) as ptp, \
                 tc.tile_pool(name="sps", bufs=4, space="PSUM") as spp, \
                 tc.tile_pool(name="ops", bufs=2, space="PSUM") as opp, \
                 tc.tile_pool(name="otp", bufs=2, space="PSUM") as otpp, \
                 tc.tile_pool(name="ots", bufs=2) as otsp, \
                 tc.tile_pool(name="ob", bufs=3) as obp, \
                 tc.tile_pool(name="rc", bufs=2) as rcp:
                for j in range(NSLOTS):
                    E = E_PAD[j]
                    mask = m_ev if j % 2 == 0 else m_od
                    qs = qt_slot[:, j * SUP : (j + 1) * SUP]
                    o_ps = opp.tile([H + 1, SUP], f32)
                    for u in range(E):
                        s0, s1 = 2 * u, 2 * u + 1
                        sa = spp.tile([128, SUP], f32, tag='s')
                        sb = spp.tile([128, SUP], f32, tag='s')
                        nc.tensor.matmul(
                            sa, kt_dup[0:64, s0 * 128 : (s0 + 1) * 128],
                            qs[0:64, :], start=True, stop=True)
                        nc.tensor.matmul(
                            sb, kt_dup[64:128, s1 * 128 : (s1 + 1) * 128],
                            qs[64:128, :], start=True, stop=True)
                        pa = ptp.tile([128, SUP], d_av, tag='p')
                        pb = ptp.tile([128, SUP], d_av, tag='p')
                        nc.scalar.activation(
                            pa, sa, mybir.ActivationFunctionType.Exp, scale=SCALE)
                        nc.scalar.activation(
                            pb, sb, mybir.ActivationFunctionType.Exp, scale=SCALE)
                        if u >= E - 2:
                            w = 2 * (u - (E - 2))
                            nc.vector.tensor_mul(pa, pa, mask[:, w, :])
                            nc.vector.tensor_mul(pb, pb, mask[:, w + 1, :])
                        nc.tensor.matmul(
                            o_ps, v_aug[:, s0, :], pa, start=(u == 0), stop=False)
                        nc.tensor.matmul(
                            o_ps, v_aug[:, s1, :], pb, start=False, stop=(u == E - 1))
                    ots = otsp.tile([H + 1, SUP], f32)
                    nc.scalar.copy(ots, o_ps)
                    for hh in range(2):
                        otps = otpp.tile([128, H + 1], f32)
                        tr(otps, ots[:, hh * 128 : (hh + 1) * 128],
                           ident[0 : H + 1, 0 : H + 1])
                        rc = rcp.tile([128, 1], f32)
                        nc.vector.reciprocal(rc, otps[:, H : H + 1])
                        ob = obp.tile([128, H], f32)
                        nc.vector.tensor_mul(
                            ob, otps[:, 0:H], rc.to_broadcast([128, H]))
                        r0 = (j * 2 + hh) * 128
                        nc.sync.dma_start(out_d[r0 : r0 + 128, :], ob)

    nc.compile()
    return nc


def get_prog():
    if "nc" not in _CACHE:
        _CACHE["nc"] = _build()
    return _CACHE["nc"]


def make_in_maps(x, Wk, Wq, Wv):
    ident = np.eye(128, dtype=np.float32)
    in_maps = []
    for c in range(NCORES):
        b, r = divmod(c, 2)
        me, mo = _masks(r)
        sched = (np.asarray(POS[r], np.int32) * SUP).reshape(1, NSLOTS)
        in_maps.append({
            "x": np.ascontiguousarray(x[b]),
            "wq": np.ascontiguousarray(Wq),
            "wk": np.ascontiguousarray(Wk),
            "wv": np.ascontiguousarray(Wv),
            "ident": ident,
            "mask_even": me,
            "mask_odd": mo,
            "sched": sched,
        })
    return in_maps


def assemble(results):
    out = np.zeros((B, T, H), np.float32)
    for c in range(NCORES):
        b, r = divmod(c, 2)
        o = results[c]["out"]
        for j in range(NSLOTS):
            p = POS[r][j]
            out[b, p * SUP : (p + 1) * SUP] = o[j * SUP : (j + 1) * SUP]
    return out


def kernel(x, Wk, Wq, Wv):
    from concourse.bass_utils import run_bass_kernel_spmd

    nc = get_prog()
    in_maps = make_in_maps(x, Wk, Wq, Wv)
    res = run_bass_kernel_spmd(nc, in_maps, core_ids=list(range(NCORES)))
    return assemble(res.results)

